# revision 9
# baseline (speedup 1.0000x reference)
"""Cartesian real SHT kernel for 8 Trainium2 NeuronCores.

Math (matching the reference):
  xf[b, k, m] = 2*pi * rfft(x, axis=-1, norm='forward')[b, k, m]   (m < MMAX)
  out_re[b, l, m] = sum_k Re(xf)[b, k, m] * W[m, l, k]
  out_im[b, l, m] = sum_k Im(xf)[b, k, m] * W[m, l, k]
  out = stack([out_re, out_im], -1)            # (1, 32, 361, 361, 2)

Device strategy (sharded over the order axis m, 46 orders/core padded):
  Stage 1 (DFT as matmul): Y[c, bk] = sum_n F[n, c] * XT[n, bk]
      F is the per-core DFT slice (c = ri*MLOC + mloc), XT = x transposed
      host-side to [n, b*K + k].  Stationary = F chunk [120, 92], moving =
      XT [120, 512] -> psum [92, 512] (float32r: 1 cycle/row at N>=512).
  Stage 1b: PE transpose of Y per (b, k-chunk) into YBIG[k, kt, mloc, 2b+ri].
  Stage 2 (Legendre): per m: psum[2b+ri, l] += YBIG[k, kt, m, :].T @ WT[m][k, l]
      with WT = weights[m].T host-side (so k is on partitions).  Moving free
      dim = L = 361 >= 256 keeps float32r at full rate.

All HBM data stays fp32 (float32r is a bit-identical view; the PE truncates
to ~FP22 mantissa internally, rel err ~1e-4 vs fp32).
"""

import math

import numpy as np

import concourse.bacc as bacc
import concourse.bass as bass
import concourse.mybir as mybir
import concourse.tile as tile
from concourse.bass_utils import run_bass_kernel_spmd

NLAT = 361
NLON = 720
NB = 32
NCORES = 8
LMAX = NLAT
MMAX = min(LMAX, NLON // 2 + 1)  # 361
MLOC = (MMAX + NCORES - 1) // NCORES  # 46 orders per core (padded)


def build_nc(B, K, L, N, mloc, nch, tw):
    """Build the single-core Bass program (same program for all cores).

    B: batch/channel count; K: nlat (contraction of stage 2); L: lmax;
    N: nlon (contraction of stage 1); mloc: orders per core; nch: number of
    partition-chunks of N (N % nch == 0, N // nch <= 128); tw: stage-1
    moving-tile width over the b*K axis.
    """
    pch = N // nch
    assert N % nch == 0 and pch <= 128
    m2 = 2 * mloc
    bk = B * K
    ktiles = [(i * 128, min(128, K - i * 128)) for i in range((K + 127) // 128)]
    nkt = len(ktiles)
    ntw = (bk + tw - 1) // tw
    # fp32r matmuls need even innermost counts on the moving operand and the
    # PSUM destination APs (s3d3_mm_fp32r_restrictions), so pad L to even.
    l2 = L + (L % 2)
    f32 = mybir.dt.float32
    f32r = mybir.dt.float32r

    nc = bacc.Bacc()
    xt = nc.declare_dram_parameter("xt", [N, bk], f32r, isOutput=False)
    fm = nc.declare_dram_parameter("fm", [N, m2], f32r, isOutput=False)
    wt = nc.declare_dram_parameter("wt", [mloc, K, l2], f32r, isOutput=False)
    iden = nc.declare_dram_parameter("iden", [m2, m2], f32r, isOutput=False)
    out = nc.declare_dram_parameter("out", [mloc, 2 * B, L], f32, isOutput=True)

    with tile.TileContext(nc) as tc:
        with (
            tc.tile_pool(name="const", bufs=1) as cpool,
            tc.tile_pool(name="xtp", bufs=3) as xpool,
            tc.tile_pool(name="wtp", bufs=10) as wpool,
            tc.tile_pool(name="osb", bufs=4) as opool,
            tc.tile_pool(name="psy", bufs=3, space="PSUM") as pypool,
            tc.tile_pool(name="pst", bufs=2, space="PSUM") as ptpool,
            tc.tile_pool(name="pso", bufs=2, space="PSUM") as popool,
        ):
            f_sb = cpool.tile([pch, nch, m2], f32r, tag="f")
            nc.sync.dma_start(f_sb, fm.rearrange("(c p) m -> p c m", p=pch))
            id_sb = cpool.tile([m2, m2], f32r, tag="iden")
            nc.sync.dma_start(id_sb, iden[:, :])
            y_sb = cpool.tile([m2, bk], f32r, tag="ysb")
            ybig = cpool.tile([128, nkt, mloc, 2 * B], f32r, tag="ybig")

            # Stage 1: Y[c, bk] = sum_n F[n, c] * XT[n, bk]
            xt_r = xt.rearrange("(c p) q -> p c q", p=pch)
            for t in range(ntw):
                c0 = t * tw
                w = min(tw, bk - c0)
                xtile = xpool.tile([pch, nch, tw], f32r, tag="x")
                nc.sync.dma_start(xtile[:, :, :w], xt_r[:, :, c0 : c0 + w])
                ps = pypool.tile([m2, tw], f32, tag="py")
                for ci in range(nch):
                    nc.tensor.matmul(
                        ps[:, :w],
                        lhsT=f_sb[:, ci, :],
                        rhs=xtile[:, ci, :w],
                        start=(ci == 0),
                        stop=(ci == nch - 1),
                    )
                nc.vector.tensor_copy(y_sb[:, c0 : c0 + w], ps[:, :w])

            # Stage 1b: transpose Y into YBIG[k, kt, mloc, 2b+ri]
            for b in range(B):
                for kti, (k0, kt) in enumerate(ktiles):
                    pt = ptpool.tile([128, m2], f32r, tag="pt")
                    nc.tensor.transpose(
                        pt[:kt, :], y_sb[:, b * K + k0 : b * K + k0 + kt], id_sb
                    )
                    nc.vector.tensor_copy(
                        ybig[:kt, kti, :, 2 * b : 2 * b + 2],
                        pt[:kt, :].rearrange("p (ri m) -> p m ri", ri=2),
                    )

            # Stage 2: per m: out[2b+ri, l] = sum_k Y[k, (b,ri)] * WT[m][k, l]
            for m in range(mloc):
                wtile = wpool.tile([128, nkt, l2], f32r, tag="w")
                for kti, (k0, kt) in enumerate(ktiles):
                    nc.sync.dma_start(wtile[:kt, kti, :], wt[m, k0 : k0 + kt, :])
                po = popool.tile([2 * B, l2], f32, tag="po")
                for kti, (k0, kt) in enumerate(ktiles):
                    nc.tensor.matmul(
                        po,
                        lhsT=ybig[:kt, kti, m, :],
                        rhs=wtile[:kt, kti, :],
                        start=(kti == 0),
                        stop=(kti == nkt - 1),
                    )
                osb = opool.tile([2 * B, l2], f32, tag="o")
                nc.vector.tensor_copy(osb, po)
                nc.sync.dma_start(out[m], osb[:, :L])
    nc.compile()
    return nc


def make_dft_slice(N, m_start, mloc, mmax):
    """F[n, c] with c = ri*mloc + j for order m = m_start + j (0 if padded).

    Re: cos(2*pi*m*n/N) * (2*pi/N);  Im: -sin(2*pi*m*n/N) * (2*pi/N).
    """
    n = np.arange(N, dtype=np.float64)
    f = np.zeros((N, 2 * mloc), dtype=np.float64)
    scale = 2.0 * np.pi / N
    for j in range(mloc):
        m = m_start + j
        if m >= mmax:
            continue
        ang = 2.0 * np.pi * m * n / N
        f[:, j] = np.cos(ang) * scale
        f[:, mloc + j] = -np.sin(ang) * scale
    return f.astype(np.float32)


_NC_CACHE = {}


def _get_nc():
    key = (NB, NLAT, LMAX, NLON, MLOC)
    if key not in _NC_CACHE:
        _NC_CACHE[key] = build_nc(NB, NLAT, LMAX, NLON, MLOC, nch=6, tw=512)
    return _NC_CACHE[key]


def prepare_in_maps(x, weights):
    x = np.asarray(x)
    weights = np.asarray(weights)
    assert x.shape == (1, NB, NLAT, NLON), x.shape
    assert weights.shape == (MMAX, LMAX, NLAT), weights.shape

    xt_full = np.ascontiguousarray(
        x.reshape(NB * NLAT, NLON).T.astype(np.float32, copy=False)
    )
    iden = np.eye(2 * MLOC, dtype=np.float32)

    in_maps = []
    for c in range(NCORES):
        m_start = c * MLOC
        mm = max(0, min(MLOC, MMAX - m_start))
        wt_c = np.zeros((MLOC, NLAT, LMAX + (LMAX % 2)), dtype=np.float32)
        if mm > 0:
            wt_c[:mm, :, :LMAX] = weights[m_start : m_start + mm].transpose(0, 2, 1)
        in_maps.append(
            {
                "xt": xt_full,
                "fm": make_dft_slice(NLON, m_start, MLOC, MMAX),
                "wt": wt_c,
                "iden": iden,
            }
        )
    return in_maps


def gather_outputs(res):
    outf = np.empty((1, NB, LMAX, MMAX, 2), dtype=np.float32)
    for c in range(NCORES):
        m_start = c * MLOC
        mm = max(0, min(MLOC, MMAX - m_start))
        if mm == 0:
            continue
        r = np.asarray(res[c]["out"]).reshape(MLOC, NB, 2, LMAX)[:mm]
        outf[0, :, :, m_start : m_start + mm, :] = r.transpose(1, 3, 0, 2)
    return outf


def kernel(x, weights):
    in_maps = prepare_in_maps(x, weights)
    nc = _get_nc()
    res = run_bass_kernel_spmd(nc, in_maps, list(range(NCORES))).results
    return gather_outputs(res)


# revision 10
# speedup vs baseline: 1.1246x; 1.1246x over previous
"""Cartesian real SHT kernel for 8 Trainium2 NeuronCores.

Math (matching the reference):
  xf[b, k, m] = 2*pi * rfft(x, axis=-1, norm='forward')[b, k, m]   (m < MMAX)
  out_re[b, l, m] = sum_k Re(xf)[b, k, m] * W[m, l, k]
  out_im[b, l, m] = sum_k Im(xf)[b, k, m] * W[m, l, k]
  out = stack([out_re, out_im], -1)            # (1, 32, 361, 361, 2)

Device strategy (sharded over the order axis m, 46 orders/core padded):
  Stage 1 (DFT as matmul): Y[c, bk] = sum_n F[n, c] * XT[n, bk]
      F is the per-core DFT slice (c = ri*MLOC + mloc), XT = x transposed
      host-side to [n, b*K + k].  Stationary = F chunk [120, 92], moving =
      XT [120, 512] -> psum [92, 512] (float32r: 1 cycle/row at N>=512).
  Stage 1b: PE transpose of Y per (b, k-chunk) into YBIG[k, kt, mloc, 2b+ri].
  Stage 2 (Legendre): per m: psum[2b+ri, l] += YBIG[k, kt, m, :].T @ WT[m][k, l]
      with WT = weights[m].T host-side (so k is on partitions).  Moving free
      dim = 362 >= 256 keeps float32r at full rate (and satisfies the
      fp32r even-innermost-count ISA restriction).

All HBM data stays fp32 (float32r is a bit-identical view; the PE truncates
to ~FP22 mantissa internally, rel err ~1e-4 vs fp32).

DMA layouts: inputs are host-packed into the exact SBUF tile images
([tile, partition, free...] contiguous slabs) so every DMA descriptor moves
>=4KB per partition — small strided descriptors cap the SDMA engines well
below line rate.
"""

import math

import numpy as np

import concourse.bacc as bacc
import concourse.bass as bass
import concourse.mybir as mybir
import concourse.tile as tile
from concourse.bass_utils import run_bass_kernel_spmd

NLAT = 361
NLON = 720
NB = 32
NCORES = 8
LMAX = NLAT
MMAX = min(LMAX, NLON // 2 + 1)  # 361
MLOC = (MMAX + NCORES - 1) // NCORES  # 46 orders per core (padded)
NCH = 6  # N partition-chunks for stage 1 (720 = 6 x 120)
TW = 512  # stage-1 moving-tile width over the b*K axis
OG = 8  # output m-group size per store DMA


def build_nc(B, K, L, N, mloc, nch, tw):
    """Build the single-core Bass program (same program for all cores)."""
    pch = N // nch
    assert N % nch == 0 and pch <= 128
    m2 = 2 * mloc
    bk = B * K
    ktiles = [(i * 128, min(128, K - i * 128)) for i in range((K + 127) // 128)]
    nkt = len(ktiles)
    ntw = (bk + tw - 1) // tw
    l2 = L + (L % 2)
    f32 = mybir.dt.float32
    f32r = mybir.dt.float32r

    nc = bacc.Bacc()
    xt = nc.declare_dram_parameter("xt", [ntw, pch, nch, tw], f32r, isOutput=False)
    fm = nc.declare_dram_parameter("fm", [N, m2], f32r, isOutput=False)
    wt = nc.declare_dram_parameter("wt", [mloc, 128, nkt, l2], f32r, isOutput=False)
    iden = nc.declare_dram_parameter("iden", [m2, m2], f32r, isOutput=False)
    out = nc.declare_dram_parameter("out", [2 * B, mloc, L], f32, isOutput=True)

    with tile.TileContext(nc) as tc:
        with (
            tc.tile_pool(name="const", bufs=1) as cpool,
            tc.tile_pool(name="xtp", bufs=3) as xpool,
            tc.tile_pool(name="wtp", bufs=6) as wpool,
            tc.tile_pool(name="osb", bufs=2) as opool,
            tc.tile_pool(name="psy", bufs=3, space="PSUM") as pypool,
            tc.tile_pool(name="pst", bufs=2, space="PSUM") as ptpool,
            tc.tile_pool(name="pso", bufs=2, space="PSUM") as popool,
        ):
            f_sb = cpool.tile([pch, nch, m2], f32r, tag="f")
            nc.sync.dma_start(f_sb, fm.rearrange("(c p) m -> p c m", p=pch))
            id_sb = cpool.tile([m2, m2], f32r, tag="iden")
            nc.sync.dma_start(id_sb, iden[:, :])
            y_sb = cpool.tile([m2, ntw * tw], f32r, tag="ysb")
            ybig = cpool.tile([128, nkt, mloc, 2 * B], f32r, tag="ybig")

            # Stage 1: Y[c, bk] = sum_n F[n, c] * XT[n, bk]
            for t in range(ntw):
                xtile = xpool.tile([pch, nch, tw], f32r, tag="x")
                nc.sync.dma_start(xtile, xt[t])
                ps = pypool.tile([m2, tw], f32, tag="py")
                for ci in range(nch):
                    nc.tensor.matmul(
                        ps,
                        lhsT=f_sb[:, ci, :],
                        rhs=xtile[:, ci, :],
                        start=(ci == 0),
                        stop=(ci == nch - 1),
                    )
                nc.vector.tensor_copy(y_sb[:, t * tw : (t + 1) * tw], ps)

            # Stage 1b: transpose Y into YBIG[k, kt, mloc, 2b+ri]
            for b in range(B):
                for kti, (k0, kt) in enumerate(ktiles):
                    pt = ptpool.tile([128, m2], f32r, tag="pt")
                    nc.tensor.transpose(
                        pt[:kt, :], y_sb[:, b * K + k0 : b * K + k0 + kt], id_sb
                    )
                    nc.vector.tensor_copy(
                        ybig[:kt, kti, :, 2 * b : 2 * b + 2],
                        pt[:kt, :].rearrange("p (ri m) -> p m ri", ri=2),
                    )

            # Stage 2: per m: out[2b+ri, l] = sum_k Y[k, (b,ri)] * WT[m][k, l]
            og = OG
            for g0 in range(0, mloc, og):
                gn = min(og, mloc - g0)
                osb = opool.tile([2 * B, og, L], f32, tag="o")
                for m in range(g0, g0 + gn):
                    wtile = wpool.tile([128, nkt, l2], f32r, tag="w")
                    nc.sync.dma_start(wtile, wt[m])
                    po = popool.tile([2 * B, l2], f32, tag="po")
                    for kti, (k0, kt) in enumerate(ktiles):
                        nc.tensor.matmul(
                            po,
                            lhsT=ybig[:kt, kti, m, :],
                            rhs=wtile[:kt, kti, :],
                            start=(kti == 0),
                            stop=(kti == nkt - 1),
                        )
                    nc.vector.tensor_copy(osb[:, m - g0, :], po[:, :L])
                nc.sync.dma_start(out[:, g0 : g0 + gn, :], osb[:, :gn, :])
    nc.compile()
    return nc


def make_dft_slice(N, m_start, mloc, mmax):
    """F[n, c] with c = ri*mloc + j for order m = m_start + j (0 if padded)."""
    n = np.arange(N, dtype=np.float64)
    f = np.zeros((N, 2 * mloc), dtype=np.float64)
    scale = 2.0 * np.pi / N
    for j in range(mloc):
        m = m_start + j
        if m >= mmax:
            continue
        ang = 2.0 * np.pi * m * n / N
        f[:, j] = np.cos(ang) * scale
        f[:, mloc + j] = -np.sin(ang) * scale
    return f.astype(np.float32)


def pack_xt(xr, nch, tw):
    """xr [bk, N] (row = b*K + k) -> [ntw, pch, nch, tw] contiguous tile slabs."""
    bk, N = xr.shape
    pch = N // nch
    ntw = (bk + tw - 1) // tw
    xtp = np.zeros((N, ntw * tw), dtype=np.float32)
    xtp[:, :bk] = xr.T
    return np.ascontiguousarray(
        xtp.reshape(nch, pch, ntw, tw).transpose(2, 1, 0, 3)
    )


def pack_wt(w_mlk, mloc, K, L):
    """w_mlk [mm, L, K] (weights slice) -> [mloc, 128, nkt, l2] k-interleaved."""
    nkt = (K + 127) // 128
    l2 = L + (L % 2)
    tmp = np.zeros((mloc, nkt * 128, l2), dtype=np.float32)
    mm = w_mlk.shape[0]
    if mm > 0:
        tmp[:mm, :K, :L] = w_mlk.transpose(0, 2, 1)
    return np.ascontiguousarray(
        tmp.reshape(mloc, nkt, 128, l2).transpose(0, 2, 1, 3)
    )


_NC_CACHE = {}


def _get_nc():
    key = (NB, NLAT, LMAX, NLON, MLOC)
    if key not in _NC_CACHE:
        _NC_CACHE[key] = build_nc(NB, NLAT, LMAX, NLON, MLOC, nch=NCH, tw=TW)
    return _NC_CACHE[key]


def prepare_in_maps(x, weights):
    x = np.asarray(x)
    weights = np.asarray(weights)
    assert x.shape == (1, NB, NLAT, NLON), x.shape
    assert weights.shape == (MMAX, LMAX, NLAT), weights.shape

    xt_full = pack_xt(
        x.reshape(NB * NLAT, NLON).astype(np.float32, copy=False), NCH, TW
    )
    iden = np.eye(2 * MLOC, dtype=np.float32)

    in_maps = []
    for c in range(NCORES):
        m_start = c * MLOC
        mm = max(0, min(MLOC, MMAX - m_start))
        in_maps.append(
            {
                "xt": xt_full,
                "fm": make_dft_slice(NLON, m_start, MLOC, MMAX),
                "wt": pack_wt(weights[m_start : m_start + mm], MLOC, NLAT, LMAX),
                "iden": iden,
            }
        )
    return in_maps


def gather_outputs(res):
    outf = np.empty((1, NB, LMAX, MMAX, 2), dtype=np.float32)
    for c in range(NCORES):
        m_start = c * MLOC
        mm = max(0, min(MLOC, MMAX - m_start))
        if mm == 0:
            continue
        # out dram is [2b+ri, m, l] -> [b, ri, m, l] -> [b, l, m, ri]
        r = np.asarray(res[c]["out"]).reshape(NB, 2, MLOC, LMAX)[:, :, :mm]
        outf[0, :, :, m_start : m_start + mm, :] = r.transpose(0, 3, 2, 1)
    return outf


def kernel(x, weights):
    in_maps = prepare_in_maps(x, weights)
    nc = _get_nc()
    res = run_bass_kernel_spmd(nc, in_maps, list(range(NCORES))).results
    return gather_outputs(res)


# revision 14
# speedup vs baseline: 1.9047x; 1.6937x over previous
"""Cartesian real SHT kernel for 8 Trainium2 NeuronCores.

Math (matching the reference):
  xf[b, k, m] = 2*pi * rfft(x, axis=-1, norm='forward')[b, k, m]   (m < MMAX)
  out_re[b, l, m] = sum_k Re(xf)[b, k, m] * W[m, l, k]
  out_im[b, l, m] = sum_k Im(xf)[b, k, m] * W[m, l, k]
  out = stack([out_re, out_im], -1)            # (1, 32, 361, 361, 2)

Device strategy (sharded over the order axis m, 46 orders/core padded):
  Stage 1 (DFT as matmul): Y[c, bk] = sum_n F[n, c] * XT[n, bk]
      F is the per-core DFT slice (c = ri*MLOC + mloc), XT = x transposed
      host-side to [n, b*K + k].  Stationary = F chunk [120, 92], moving =
      XT [120, 512] -> psum [92, 512] (float32r: 1 cycle/row at N>=512).
  Stage 1b: PE transpose of Y per (b, k-chunk) into YBIG[k, kt, mloc, 2b+ri].
  Stage 2 (Legendre): per m: psum[2b+ri, l] += YBIG[k, kt, m, :].T @ WT[m][k, l]
      with WT = weights[m].T host-side (so k is on partitions).  Moving free
      dim = 362 >= 256 keeps float32r at full rate (and satisfies the
      fp32r even-innermost-count ISA restriction).

All HBM data stays fp32 (float32r is a bit-identical view; the PE truncates
to ~FP22 mantissa internally, rel err ~1e-4 vs fp32).

DMA layouts: inputs are host-packed into the exact SBUF tile images
([tile, partition, free...] contiguous slabs) so every DMA descriptor moves
>=4KB per partition — small strided descriptors cap the SDMA engines well
below line rate.
"""

import math

import ml_dtypes
import numpy as np

import concourse.bacc as bacc
import concourse.bass as bass
import concourse.mybir as mybir
import concourse.tile as tile
from concourse.bass_utils import run_bass_kernel_spmd

NLAT = 361
NLON = 720
NB = 32
NCORES = 8
LMAX = NLAT
MMAX = min(LMAX, NLON // 2 + 1)  # 361
MLOC = (MMAX + NCORES - 1) // NCORES  # 46 orders per core (padded)
NCH = 6  # N partition-chunks for stage 1 (720 = 6 x 120)
TW = 512  # stage-1 moving-tile width over the b*K axis
OG = 8  # output m-group size per store DMA

# Matmul-operand dtype in HBM/SBUF: "bf16" (half the DMA bytes, rel err
# ~4e-3) or "f32r" (fp32 bytes, FP22 multiplies, rel err ~2e-4).  PSUM
# accumulation and the output are fp32 either way.
DATA_DTYPE = "bf16"

_NP_DT = {"bf16": ml_dtypes.bfloat16, "f32r": np.float32}


def build_nc(B, K, L, N, mloc, nch, tw, ddt=DATA_DTYPE):
    """Build the single-core Bass program (same program for all cores)."""
    pch = N // nch
    assert N % nch == 0 and pch <= 128
    m2 = 2 * mloc
    bk = B * K
    ktiles = [(i * 128, min(128, K - i * 128)) for i in range((K + 127) // 128)]
    nkt = len(ktiles)
    ntw = (bk + tw - 1) // tw
    l2 = L + (L % 2)
    f32 = mybir.dt.float32
    f32r = {"bf16": mybir.dt.bfloat16, "f32r": mybir.dt.float32r}[ddt]

    nc = bacc.Bacc()
    xt = nc.declare_dram_parameter("xt", [ntw, pch, nch, tw], f32r, isOutput=False)
    fm = nc.declare_dram_parameter("fm", [N, m2], f32r, isOutput=False)
    wt = nc.declare_dram_parameter("wt", [mloc, 128, nkt, l2], f32r, isOutput=False)
    iden = nc.declare_dram_parameter("iden", [m2, m2], f32r, isOutput=False)
    out = nc.declare_dram_parameter("out", [2 * B, mloc, L], f32, isOutput=True)

    with tile.TileContext(nc) as tc:
        with (
            tc.tile_pool(name="const", bufs=1) as cpool,
            tc.tile_pool(name="xtp", bufs=3) as xpool,
            tc.tile_pool(name="wtp", bufs=6) as wpool,
            tc.tile_pool(name="osb", bufs=2) as opool,
            tc.tile_pool(name="psy", bufs=3, space="PSUM") as pypool,
            tc.tile_pool(name="pst", bufs=2, space="PSUM") as ptpool,
            tc.tile_pool(name="pso", bufs=2, space="PSUM") as popool,
        ):
            f_sb = cpool.tile([pch, nch, m2], f32r, tag="f")
            nc.sync.dma_start(f_sb, fm.rearrange("(c p) m -> p c m", p=pch))
            id_sb = cpool.tile([m2, m2], f32r, tag="iden")
            nc.sync.dma_start(id_sb, iden[:, :])
            y_sb = cpool.tile([m2, ntw * tw], f32r, tag="ysb")
            ybig = cpool.tile([128, nkt, mloc, 2 * B], f32r, tag="ybig")

            # Stage 1: Y[c, bk] = sum_n F[n, c] * XT[n, bk]
            for t in range(ntw):
                xtile = xpool.tile([pch, nch, tw], f32r, tag="x")
                nc.sync.dma_start(xtile, xt[t])
                ps = pypool.tile([m2, tw], f32, tag="py")
                for ci in range(nch):
                    nc.tensor.matmul(
                        ps,
                        lhsT=f_sb[:, ci, :],
                        rhs=xtile[:, ci, :],
                        start=(ci == 0),
                        stop=(ci == nch - 1),
                    )
                nc.vector.tensor_copy(y_sb[:, t * tw : (t + 1) * tw], ps)

            # Stage 1b: transpose Y into YBIG[k, kt, mloc, 2b+ri]
            for b in range(B):
                for kti, (k0, kt) in enumerate(ktiles):
                    pt = ptpool.tile([128, m2], f32r, tag="pt")
                    nc.tensor.transpose(
                        pt[:kt, :], y_sb[:, b * K + k0 : b * K + k0 + kt], id_sb
                    )
                    nc.vector.tensor_copy(
                        ybig[:kt, kti, :, 2 * b : 2 * b + 2],
                        pt[:kt, :].rearrange("p (ri m) -> p m ri", ri=2),
                    )

            # Stage 2: per m: out[2b+ri, l] = sum_k Y[k, (b,ri)] * WT[m][k, l]
            og = OG
            for g0 in range(0, mloc, og):
                gn = min(og, mloc - g0)
                osb = opool.tile([2 * B, og, L], f32, tag="o")
                for m in range(g0, g0 + gn):
                    wtile = wpool.tile([128, nkt, l2], f32r, tag="w")
                    nc.sync.dma_start(wtile, wt[m])
                    po = popool.tile([2 * B, l2], f32, tag="po")
                    for kti, (k0, kt) in enumerate(ktiles):
                        nc.tensor.matmul(
                            po,
                            lhsT=ybig[:kt, kti, m, :],
                            rhs=wtile[:kt, kti, :],
                            start=(kti == 0),
                            stop=(kti == nkt - 1),
                        )
                    nc.vector.tensor_copy(osb[:, m - g0, :], po[:, :L])
                nc.sync.dma_start(out[:, g0 : g0 + gn, :], osb[:, :gn, :])
    nc.compile()
    return nc


def make_dft_slice(N, m_start, mloc, mmax, ddt=DATA_DTYPE):
    """F[n, c] with c = ri*mloc + j for order m = m_start + j (0 if padded)."""
    n = np.arange(N, dtype=np.float64)
    f = np.zeros((N, 2 * mloc), dtype=np.float64)
    scale = 2.0 * np.pi / N
    for j in range(mloc):
        m = m_start + j
        if m >= mmax:
            continue
        ang = 2.0 * np.pi * m * n / N
        f[:, j] = np.cos(ang) * scale
        f[:, mloc + j] = -np.sin(ang) * scale
    return f.astype(_NP_DT[ddt])


def pack_xt(xr, nch, tw, ddt=DATA_DTYPE):
    """xr [bk, N] (row = b*K + k) -> [ntw, pch, nch, tw] contiguous tile slabs."""
    bk, N = xr.shape
    pch = N // nch
    ntw = (bk + tw - 1) // tw
    xtp = np.zeros((N, ntw * tw), dtype=_NP_DT[ddt])
    xtp[:, :bk] = xr.T
    return np.ascontiguousarray(
        xtp.reshape(nch, pch, ntw, tw).transpose(2, 1, 0, 3)
    )


def pack_wt(w_mlk, mloc, K, L, ddt=DATA_DTYPE):
    """w_mlk [mm, L, K] (weights slice) -> [mloc, 128, nkt, l2] k-interleaved."""
    nkt = (K + 127) // 128
    l2 = L + (L % 2)
    tmp = np.zeros((mloc, nkt * 128, l2), dtype=_NP_DT[ddt])
    mm = w_mlk.shape[0]
    if mm > 0:
        tmp[:mm, :K, :L] = w_mlk.transpose(0, 2, 1).astype(_NP_DT[ddt])
    return np.ascontiguousarray(
        tmp.reshape(mloc, nkt, 128, l2).transpose(0, 2, 1, 3)
    )


_NC_CACHE = {}


def _get_nc():
    key = (NB, NLAT, LMAX, NLON, MLOC)
    if key not in _NC_CACHE:
        _NC_CACHE[key] = build_nc(NB, NLAT, LMAX, NLON, MLOC, nch=NCH, tw=TW)
    return _NC_CACHE[key]


def prepare_in_maps(x, weights):
    x = np.asarray(x)
    weights = np.asarray(weights)
    assert x.shape == (1, NB, NLAT, NLON), x.shape
    assert weights.shape == (MMAX, LMAX, NLAT), weights.shape

    xt_full = pack_xt(
        x.reshape(NB * NLAT, NLON).astype(np.float32, copy=False), NCH, TW
    )
    iden = np.eye(2 * MLOC, dtype=_NP_DT[DATA_DTYPE])

    in_maps = []
    for c in range(NCORES):
        m_start = c * MLOC
        mm = max(0, min(MLOC, MMAX - m_start))
        in_maps.append(
            {
                "xt": xt_full,
                "fm": make_dft_slice(NLON, m_start, MLOC, MMAX),
                "wt": pack_wt(weights[m_start : m_start + mm], MLOC, NLAT, LMAX),
                "iden": iden,
            }
        )
    return in_maps


def gather_outputs(res):
    outf = np.empty((1, NB, LMAX, MMAX, 2), dtype=np.float32)
    for c in range(NCORES):
        m_start = c * MLOC
        mm = max(0, min(MLOC, MMAX - m_start))
        if mm == 0:
            continue
        # out dram is [2b+ri, m, l] -> [b, ri, m, l] -> [b, l, m, ri]
        r = np.asarray(res[c]["out"]).reshape(NB, 2, MLOC, LMAX)[:, :, :mm]
        outf[0, :, :, m_start : m_start + mm, :] = r.transpose(0, 3, 2, 1)
    return outf


def kernel(x, weights):
    in_maps = prepare_in_maps(x, weights)
    nc = _get_nc()
    res = run_bass_kernel_spmd(nc, in_maps, list(range(NCORES))).results
    return gather_outputs(res)


# revision 15
# speedup vs baseline: 2.0473x; 1.0748x over previous
"""Cartesian real SHT kernel for 8 Trainium2 NeuronCores.

Math (matching the reference):
  xf[b, k, m] = 2*pi * rfft(x, axis=-1, norm='forward')[b, k, m]   (m < MMAX)
  out_re[b, l, m] = sum_k Re(xf)[b, k, m] * W[m, l, k]
  out_im[b, l, m] = sum_k Im(xf)[b, k, m] * W[m, l, k]
  out = stack([out_re, out_im], -1)            # (1, 32, 361, 361, 2)

Device strategy (sharded over the order axis m, 46 orders/core padded):
  Stage 1 (DFT as matmul): Y[c, bk] = sum_n F[n, c] * XT[n, bk]
      F is the per-core DFT slice (c = ri*MLOC + mloc), XT = x transposed
      host-side to [n, b*K + k].  Stationary = F chunk [120, 92], moving =
      XT [120, 512] -> psum [92, 512] (float32r: 1 cycle/row at N>=512).
  Stage 1b: PE transpose of Y per (b, k-chunk) into YBIG[k, kt, mloc, 2b+ri].
  Stage 2 (Legendre): per m: psum[2b+ri, l] += YBIG[k, kt, m, :].T @ WT[m][k, l]
      with WT = weights[m].T host-side (so k is on partitions).  Moving free
      dim = 362 >= 256 keeps float32r at full rate (and satisfies the
      fp32r even-innermost-count ISA restriction).

All HBM data stays fp32 (float32r is a bit-identical view; the PE truncates
to ~FP22 mantissa internally, rel err ~1e-4 vs fp32).

DMA layouts: inputs are host-packed into the exact SBUF tile images
([tile, partition, free...] contiguous slabs) so every DMA descriptor moves
>=4KB per partition — small strided descriptors cap the SDMA engines well
below line rate.
"""

import math

import ml_dtypes
import numpy as np

import concourse.bacc as bacc
import concourse.bass as bass
import concourse.mybir as mybir
import concourse.tile as tile
from concourse.bass_utils import run_bass_kernel_spmd

NLAT = 361
NLON = 720
NB = 32
NCORES = 8
LMAX = NLAT
MMAX = min(LMAX, NLON // 2 + 1)  # 361
MLOC = (MMAX + NCORES - 1) // NCORES  # 46 orders per core (padded)
NCH = 6  # N partition-chunks for stage 1 (720 = 6 x 120)
TW = 512  # stage-1 moving-tile width over the b*K axis
OG = 8  # output m-group size per store DMA

# Matmul-operand dtype in HBM/SBUF: "bf16" (half the DMA bytes, rel err
# ~4e-3) or "f32r" (fp32 bytes, FP22 multiplies, rel err ~2e-4).  PSUM
# accumulation and the output are fp32 either way.
DATA_DTYPE = "bf16"

_NP_DT = {"bf16": ml_dtypes.bfloat16, "f32r": np.float32}


def build_nc(B, K, L, N, mloc, nch, tw, ddt=DATA_DTYPE):
    """Build the single-core Bass program (same program for all cores)."""
    pch = N // nch
    assert N % nch == 0 and pch <= 128
    m2 = 2 * mloc
    bk = B * K
    ktiles = [(i * 128, min(128, K - i * 128)) for i in range((K + 127) // 128)]
    nkt = len(ktiles)
    ntw = (bk + tw - 1) // tw
    l2 = L + (L % 2)
    f32 = mybir.dt.float32
    f32r = {"bf16": mybir.dt.bfloat16, "f32r": mybir.dt.float32r}[ddt]

    nc = bacc.Bacc()
    xt = nc.declare_dram_parameter("xt", [ntw, pch, nch, tw], f32r, isOutput=False)
    fm = nc.declare_dram_parameter("fm", [N, m2], f32r, isOutput=False)
    wt = nc.declare_dram_parameter("wt", [mloc, 128, nkt, l2], f32r, isOutput=False)
    iden = nc.declare_dram_parameter("iden", [m2, m2], f32r, isOutput=False)
    out = nc.declare_dram_parameter("out", [2 * B, mloc, L], f32, isOutput=True)

    with tile.TileContext(nc) as tc:
        with (
            tc.tile_pool(name="const", bufs=1) as cpool,
            tc.tile_pool(name="xtp", bufs=5) as xpool,
            tc.tile_pool(name="wtp", bufs=28 if ddt == "bf16" else 8) as wpool,
            tc.tile_pool(name="osb", bufs=3) as opool,
            tc.tile_pool(name="psy", bufs=3, space="PSUM") as pypool,
            tc.tile_pool(name="pst", bufs=2, space="PSUM") as ptpool,
            tc.tile_pool(name="pso", bufs=2, space="PSUM") as popool,
        ):
            f_sb = cpool.tile([pch, nch, m2], f32r, tag="f")
            nc.sync.dma_start(f_sb, fm.rearrange("(c p) m -> p c m", p=pch))
            id_sb = cpool.tile([m2, m2], f32r, tag="iden")
            nc.sync.dma_start(id_sb, iden[:, :])
            y_sb = cpool.tile([m2, ntw * tw], f32r, tag="ysb")
            ybig = cpool.tile([128, nkt, mloc, 2 * B], f32r, tag="ybig")

            # Stage 1: Y[c, bk] = sum_n F[n, c] * XT[n, bk]
            for t in range(ntw):
                xtile = xpool.tile([pch, nch, tw], f32r, tag="x")
                nc.sync.dma_start(xtile, xt[t])
                ps = pypool.tile([m2, tw], f32, tag="py")
                for ci in range(nch):
                    nc.tensor.matmul(
                        ps,
                        lhsT=f_sb[:, ci, :],
                        rhs=xtile[:, ci, :],
                        start=(ci == 0),
                        stop=(ci == nch - 1),
                    )
                nc.vector.tensor_copy(y_sb[:, t * tw : (t + 1) * tw], ps)

            # Stage 1b: transpose Y into YBIG[k, kt, mloc, 2b+ri]
            for b in range(B):
                for kti, (k0, kt) in enumerate(ktiles):
                    pt = ptpool.tile([128, m2], f32r, tag="pt")
                    nc.tensor.transpose(
                        pt[:kt, :], y_sb[:, b * K + k0 : b * K + k0 + kt], id_sb
                    )
                    nc.vector.tensor_copy(
                        ybig[:kt, kti, :, 2 * b : 2 * b + 2],
                        pt[:kt, :].rearrange("p (ri m) -> p m ri", ri=2),
                    )

            # Stage 2: per m: out[2b+ri, l] = sum_k Y[k, (b,ri)] * WT[m][k, l]
            og = OG
            for g0 in range(0, mloc, og):
                gn = min(og, mloc - g0)
                osb = opool.tile([2 * B, og, L], f32, tag="o")
                for m in range(g0, g0 + gn):
                    wtile = wpool.tile([128, nkt, l2], f32r, tag="w")
                    nc.sync.dma_start(wtile, wt[m])
                    po = popool.tile([2 * B, l2], f32, tag="po")
                    for kti, (k0, kt) in enumerate(ktiles):
                        nc.tensor.matmul(
                            po,
                            lhsT=ybig[:kt, kti, m, :],
                            rhs=wtile[:kt, kti, :],
                            start=(kti == 0),
                            stop=(kti == nkt - 1),
                        )
                    nc.vector.tensor_copy(osb[:, m - g0, :], po[:, :L])
                nc.sync.dma_start(out[:, g0 : g0 + gn, :], osb[:, :gn, :])
    nc.compile()
    return nc


def make_dft_slice(N, m_start, mloc, mmax, ddt=DATA_DTYPE):
    """F[n, c] with c = ri*mloc + j for order m = m_start + j (0 if padded)."""
    n = np.arange(N, dtype=np.float64)
    f = np.zeros((N, 2 * mloc), dtype=np.float64)
    scale = 2.0 * np.pi / N
    for j in range(mloc):
        m = m_start + j
        if m >= mmax:
            continue
        ang = 2.0 * np.pi * m * n / N
        f[:, j] = np.cos(ang) * scale
        f[:, mloc + j] = -np.sin(ang) * scale
    return f.astype(_NP_DT[ddt])


def pack_xt(xr, nch, tw, ddt=DATA_DTYPE):
    """xr [bk, N] (row = b*K + k) -> [ntw, pch, nch, tw] contiguous tile slabs."""
    bk, N = xr.shape
    pch = N // nch
    ntw = (bk + tw - 1) // tw
    xtp = np.zeros((N, ntw * tw), dtype=_NP_DT[ddt])
    xtp[:, :bk] = xr.T
    return np.ascontiguousarray(
        xtp.reshape(nch, pch, ntw, tw).transpose(2, 1, 0, 3)
    )


def pack_wt(w_mlk, mloc, K, L, ddt=DATA_DTYPE):
    """w_mlk [mm, L, K] (weights slice) -> [mloc, 128, nkt, l2] k-interleaved."""
    nkt = (K + 127) // 128
    l2 = L + (L % 2)
    tmp = np.zeros((mloc, nkt * 128, l2), dtype=_NP_DT[ddt])
    mm = w_mlk.shape[0]
    if mm > 0:
        tmp[:mm, :K, :L] = w_mlk.transpose(0, 2, 1).astype(_NP_DT[ddt])
    return np.ascontiguousarray(
        tmp.reshape(mloc, nkt, 128, l2).transpose(0, 2, 1, 3)
    )


_NC_CACHE = {}


def _get_nc():
    key = (NB, NLAT, LMAX, NLON, MLOC)
    if key not in _NC_CACHE:
        _NC_CACHE[key] = build_nc(NB, NLAT, LMAX, NLON, MLOC, nch=NCH, tw=TW)
    return _NC_CACHE[key]


def prepare_in_maps(x, weights):
    x = np.asarray(x)
    weights = np.asarray(weights)
    assert x.shape == (1, NB, NLAT, NLON), x.shape
    assert weights.shape == (MMAX, LMAX, NLAT), weights.shape

    xt_full = pack_xt(
        x.reshape(NB * NLAT, NLON).astype(np.float32, copy=False), NCH, TW
    )
    iden = np.eye(2 * MLOC, dtype=_NP_DT[DATA_DTYPE])

    in_maps = []
    for c in range(NCORES):
        m_start = c * MLOC
        mm = max(0, min(MLOC, MMAX - m_start))
        in_maps.append(
            {
                "xt": xt_full,
                "fm": make_dft_slice(NLON, m_start, MLOC, MMAX),
                "wt": pack_wt(weights[m_start : m_start + mm], MLOC, NLAT, LMAX),
                "iden": iden,
            }
        )
    return in_maps


def gather_outputs(res):
    outf = np.empty((1, NB, LMAX, MMAX, 2), dtype=np.float32)
    for c in range(NCORES):
        m_start = c * MLOC
        mm = max(0, min(MLOC, MMAX - m_start))
        if mm == 0:
            continue
        # out dram is [2b+ri, m, l] -> [b, ri, m, l] -> [b, l, m, ri]
        r = np.asarray(res[c]["out"]).reshape(NB, 2, MLOC, LMAX)[:, :, :mm]
        outf[0, :, :, m_start : m_start + mm, :] = r.transpose(0, 3, 2, 1)
    return outf


def kernel(x, weights):
    in_maps = prepare_in_maps(x, weights)
    nc = _get_nc()
    res = run_bass_kernel_spmd(nc, in_maps, list(range(NCORES))).results
    return gather_outputs(res)


# revision 17
# speedup vs baseline: 2.0532x; 1.0029x over previous
"""Cartesian real SHT kernel for 8 Trainium2 NeuronCores.

Math (matching the reference):
  xf[b, k, m] = 2*pi * rfft(x, axis=-1, norm='forward')[b, k, m]   (m < MMAX)
  out_re[b, l, m] = sum_k Re(xf)[b, k, m] * W[m, l, k]
  out_im[b, l, m] = sum_k Im(xf)[b, k, m] * W[m, l, k]
  out = stack([out_re, out_im], -1)            # (1, 32, 361, 361, 2)

Device strategy (sharded over the order axis m, 46 orders/core padded):
  Stage 1 (DFT as matmul): Y[c, bk] = sum_n F[n, c] * XT[n, bk]
      F is the per-core DFT slice (c = ri*MLOC + mloc), XT = x transposed
      host-side to [n, b*K + k].  Stationary = F chunk [120, 92], moving =
      XT [120, 512] -> psum [92, 512] (float32r: 1 cycle/row at N>=512).
  Stage 1b: PE transpose of Y per (b, k-chunk) into YBIG[k, kt, mloc, 2b+ri].
  Stage 2 (Legendre): per m: psum[2b+ri, l] += YBIG[k, kt, m, :].T @ WT[m][k, l]
      with WT = weights[m].T host-side (so k is on partitions).  Moving free
      dim = 362 >= 256 keeps float32r at full rate (and satisfies the
      fp32r even-innermost-count ISA restriction).

All HBM data stays fp32 (float32r is a bit-identical view; the PE truncates
to ~FP22 mantissa internally, rel err ~1e-4 vs fp32).

DMA layouts: inputs are host-packed into the exact SBUF tile images
([tile, partition, free...] contiguous slabs) so every DMA descriptor moves
>=4KB per partition — small strided descriptors cap the SDMA engines well
below line rate.
"""

import math

import ml_dtypes
import numpy as np

import concourse.bacc as bacc
import concourse.bass as bass
import concourse.mybir as mybir
import concourse.tile as tile
from concourse.bass_utils import run_bass_kernel_spmd

NLAT = 361
NLON = 720
NB = 32
NCORES = 8
LMAX = NLAT
MMAX = min(LMAX, NLON // 2 + 1)  # 361
MLOC = (MMAX + NCORES - 1) // NCORES  # 46 orders per core (padded)
NCH = 6  # N partition-chunks for stage 1 (720 = 6 x 120)
TW = 512  # stage-1 moving-tile width over the b*K axis
OG = 8  # output m-group size per store DMA

# Matmul-operand dtype in HBM/SBUF: "bf16" (half the DMA bytes, rel err
# ~4e-3) or "f32r" (fp32 bytes, FP22 multiplies, rel err ~2e-4).  PSUM
# accumulation and the output are fp32 either way.
DATA_DTYPE = "bf16"

_NP_DT = {"bf16": ml_dtypes.bfloat16, "f32r": np.float32}


def build_nc(B, K, L, N, mloc, nch, tw, ddt=DATA_DTYPE):
    """Build the single-core Bass program (same program for all cores)."""
    pch = N // nch
    assert N % nch == 0 and pch <= 128
    m2 = 2 * mloc
    bk = B * K
    ktiles = [(i * 128, min(128, K - i * 128)) for i in range((K + 127) // 128)]
    nkt = len(ktiles)
    ntw = (bk + tw - 1) // tw
    l2 = L + (L % 2)
    f32 = mybir.dt.float32
    f32r = {"bf16": mybir.dt.bfloat16, "f32r": mybir.dt.float32r}[ddt]

    nc = bacc.Bacc()
    xt = nc.declare_dram_parameter("xt", [ntw, pch, nch, tw], f32r, isOutput=False)
    fm = nc.declare_dram_parameter("fm", [N, m2], f32r, isOutput=False)
    wt = nc.declare_dram_parameter("wt", [mloc, 128, nkt, l2], f32r, isOutput=False)
    iden = nc.declare_dram_parameter("iden", [m2, m2], f32r, isOutput=False)
    out = nc.declare_dram_parameter("out", [2 * B, mloc, L], f32, isOutput=True)

    with tile.TileContext(nc) as tc:
        with (
            tc.tile_pool(name="const", bufs=1) as cpool,
            tc.tile_pool(name="xtp", bufs=5) as xpool,
            tc.tile_pool(name="wtp", bufs=28 if ddt == "bf16" else 8) as wpool,
            tc.tile_pool(name="osb", bufs=3) as opool,
            tc.tile_pool(name="psy", bufs=2, space="PSUM") as pypool,
            tc.tile_pool(name="pst", bufs=3, space="PSUM") as ptpool,
            tc.tile_pool(name="pso", bufs=2, space="PSUM") as popool,
        ):
            f_sb = cpool.tile([pch, nch, m2], f32r, tag="f")
            nc.sync.dma_start(f_sb, fm.rearrange("(c p) m -> p c m", p=pch))
            id_sb = cpool.tile([m2, m2], f32r, tag="iden")
            nc.sync.dma_start(id_sb, iden[:, :])
            # Per-b Y tiles so each b's transposes unblock as soon as the
            # stage-1 tiles covering its columns land (fills PE gaps).
            yb = [
                cpool.tile([m2, K], f32r, tag=f"yb{b}", name=f"yb{b}")
                for b in range(B)
            ]
            ybig = cpool.tile([128, nkt, mloc, 2 * B], f32r, tag="ybig")

            # Stage 1: Y[c, bk] = sum_n F[n, c] * XT[n, bk]
            for t in range(ntw):
                xtile = xpool.tile([pch, nch, tw], f32r, tag="x")
                nc.sync.dma_start(xtile, xt[t])
                ps = pypool.tile([m2, tw], f32, tag="py")
                for ci in range(nch):
                    nc.tensor.matmul(
                        ps,
                        lhsT=f_sb[:, ci, :],
                        rhs=xtile[:, ci, :],
                        start=(ci == 0),
                        stop=(ci == nch - 1),
                    )
                c0, c1 = t * tw, min((t + 1) * tw, bk)
                b0, b1 = c0 // K, (c1 - 1) // K
                for b in range(b0, b1 + 1):
                    lo, hi = max(c0, b * K), min(c1, (b + 1) * K)
                    nc.vector.tensor_copy(
                        yb[b][:, lo - b * K : hi - b * K], ps[:, lo - c0 : hi - c0]
                    )

            # Stage 1b: transpose Y into YBIG[k, kt, mloc, 2b+ri]
            for b in range(B):
                for kti, (k0, kt) in enumerate(ktiles):
                    pt = ptpool.tile([128, m2], f32r, tag="pt")
                    nc.tensor.transpose(
                        pt[:kt, :], yb[b][:, k0 : k0 + kt], id_sb
                    )
                    nc.vector.tensor_copy(
                        ybig[:kt, kti, :, 2 * b : 2 * b + 2],
                        pt[:kt, :].rearrange("p (ri m) -> p m ri", ri=2),
                    )

            # Stage 2: per m: out[2b+ri, l] = sum_k Y[k, (b,ri)] * WT[m][k, l]
            og = OG
            for g0 in range(0, mloc, og):
                gn = min(og, mloc - g0)
                osb = opool.tile([2 * B, og, L], f32, tag="o")
                for m in range(g0, g0 + gn):
                    wtile = wpool.tile([128, nkt, l2], f32r, tag="w")
                    nc.sync.dma_start(wtile, wt[m])
                    po = popool.tile([2 * B, l2], f32, tag="po")
                    for kti, (k0, kt) in enumerate(ktiles):
                        nc.tensor.matmul(
                            po,
                            lhsT=ybig[:kt, kti, m, :],
                            rhs=wtile[:kt, kti, :],
                            start=(kti == 0),
                            stop=(kti == nkt - 1),
                        )
                    nc.vector.tensor_copy(osb[:, m - g0, :], po[:, :L])
                nc.sync.dma_start(out[:, g0 : g0 + gn, :], osb[:, :gn, :])
    nc.compile()
    return nc


def make_dft_slice(N, m_start, mloc, mmax, ddt=DATA_DTYPE):
    """F[n, c] with c = ri*mloc + j for order m = m_start + j (0 if padded)."""
    n = np.arange(N, dtype=np.float64)
    f = np.zeros((N, 2 * mloc), dtype=np.float64)
    scale = 2.0 * np.pi / N
    for j in range(mloc):
        m = m_start + j
        if m >= mmax:
            continue
        ang = 2.0 * np.pi * m * n / N
        f[:, j] = np.cos(ang) * scale
        f[:, mloc + j] = -np.sin(ang) * scale
    return f.astype(_NP_DT[ddt])


def pack_xt(xr, nch, tw, ddt=DATA_DTYPE):
    """xr [bk, N] (row = b*K + k) -> [ntw, pch, nch, tw] contiguous tile slabs."""
    bk, N = xr.shape
    pch = N // nch
    ntw = (bk + tw - 1) // tw
    xtp = np.zeros((N, ntw * tw), dtype=_NP_DT[ddt])
    xtp[:, :bk] = xr.T
    return np.ascontiguousarray(
        xtp.reshape(nch, pch, ntw, tw).transpose(2, 1, 0, 3)
    )


def pack_wt(w_mlk, mloc, K, L, ddt=DATA_DTYPE):
    """w_mlk [mm, L, K] (weights slice) -> [mloc, 128, nkt, l2] k-interleaved."""
    nkt = (K + 127) // 128
    l2 = L + (L % 2)
    tmp = np.zeros((mloc, nkt * 128, l2), dtype=_NP_DT[ddt])
    mm = w_mlk.shape[0]
    if mm > 0:
        tmp[:mm, :K, :L] = w_mlk.transpose(0, 2, 1).astype(_NP_DT[ddt])
    return np.ascontiguousarray(
        tmp.reshape(mloc, nkt, 128, l2).transpose(0, 2, 1, 3)
    )


_NC_CACHE = {}


def _get_nc():
    key = (NB, NLAT, LMAX, NLON, MLOC)
    if key not in _NC_CACHE:
        _NC_CACHE[key] = build_nc(NB, NLAT, LMAX, NLON, MLOC, nch=NCH, tw=TW)
    return _NC_CACHE[key]


def prepare_in_maps(x, weights):
    x = np.asarray(x)
    weights = np.asarray(weights)
    assert x.shape == (1, NB, NLAT, NLON), x.shape
    assert weights.shape == (MMAX, LMAX, NLAT), weights.shape

    xt_full = pack_xt(
        x.reshape(NB * NLAT, NLON).astype(np.float32, copy=False), NCH, TW
    )
    iden = np.eye(2 * MLOC, dtype=_NP_DT[DATA_DTYPE])

    in_maps = []
    for c in range(NCORES):
        m_start = c * MLOC
        mm = max(0, min(MLOC, MMAX - m_start))
        in_maps.append(
            {
                "xt": xt_full,
                "fm": make_dft_slice(NLON, m_start, MLOC, MMAX),
                "wt": pack_wt(weights[m_start : m_start + mm], MLOC, NLAT, LMAX),
                "iden": iden,
            }
        )
    return in_maps


def gather_outputs(res):
    outf = np.empty((1, NB, LMAX, MMAX, 2), dtype=np.float32)
    for c in range(NCORES):
        m_start = c * MLOC
        mm = max(0, min(MLOC, MMAX - m_start))
        if mm == 0:
            continue
        # out dram is [2b+ri, m, l] -> [b, ri, m, l] -> [b, l, m, ri]
        r = np.asarray(res[c]["out"]).reshape(NB, 2, MLOC, LMAX)[:, :, :mm]
        outf[0, :, :, m_start : m_start + mm, :] = r.transpose(0, 3, 2, 1)
    return outf


def kernel(x, weights):
    in_maps = prepare_in_maps(x, weights)
    nc = _get_nc()
    res = run_bass_kernel_spmd(nc, in_maps, list(range(NCORES))).results
    return gather_outputs(res)
